# revision 19
# baseline (speedup 1.0000x reference)
"""Trainium2 Bass kernel for nn_ClusterLoss (segment_reduce family).

Reference computes:
    loss = w0*omega_mean + (w1*omega_between + w2*omega_within) / bs
with (w0, w1, w2) = (1.0, 0.5, 0.5).

Algebra: with S_c the per-group column sums, t the total column sum vector,
B = sum_c ||S_c||^2 / max(m_c, 1):
    omega_within  = omega_mean - B
    omega_between = B - ||t||^2 / n
Since w1 == w2, B cancels exactly:
    loss = omega_mean + 0.5*(omega_mean - ||t||^2/n)/bs
so only one streaming pass over W is needed: sum of squares + row sums.
group_ids does not influence the result.

Device plan (per core, column shard [1024, 6250] f32, 8 chunks of 128 rows):
  - chunk-resident SBUF tiles [128 x 6250], filled by 1250-col DMAs
    (descending DMA widths on the last chunk so the stream tail is fine)
  - VectorE (DVE): reduce_sum along free axis -> per-partition row sums
  - ScalarE (ACT): Square activation with accum_out -> per-partition sq sums
  Engine op splits are aligned with DMA arrivals on the last chunk so the
  serial backlog after the final byte is minimal.
  - stats for chunks 0..6 are DMA'd out mid-stream; only the last chunk's
    small stats DMA sits on the critical tail.
Host reduces the tiny [128, NSLOT] per-core stats in float64.
"""

import numpy as np

D = 1024
N_CLASSES = 50000
N_CORES = 8
P = 128
COLS = N_CLASSES // N_CORES      # 6250 columns per core
N_CHUNKS = D // P                # 8 partition chunks

BULK_DMA = (1250, 1250, 1250, 1250, 1250)
TAIL_DMA = (1250, 1250, 1250, 625, 625, 625, 625)
BULK_ROW_OPS = (2500, 2500, 1250)
BULK_SQ_OPS = (2500, 2500, 1250)
TAIL_ROW_OPS = TAIL_DMA                       # DVE rowsums, aligned to DMAs
TAIL_SQ_ACT = (1250, 1250, 1250, 1250, 1250)  # ACT squares: fewer, coarser ops
# (ACT pays ~0.37us fixed cost per accum op, so five 1250-wide ops beat
# DMA-aligned ops in the tail; sq op k is ready once its two 625 DMAs land)
# NOTE: offloading tail squares to DVE via tensor_tensor_reduce looked good in
# the cost model but crashes real TRN2 (NRT_EXEC_UNIT_UNRECOVERABLE) in this
# toolchain even in its canonical broadcast-out form, so all squares stay on ACT.


def _widths_to_ranges(widths, off=0):
    out = []
    for w in widths:
        out.append((off, w))
        off += w
    return out


def _slots():
    """stats-column layout:
      section A (bulk, chunks 0..6): rows then sqs, BULK_OPS splits
      section B (chunk 7): rows (TAIL_ROW_OPS) | sqs (TAIL_SQ_ACT)
    Returns slots: slots[i] = (kind, chunk, off, width)."""
    slots = []
    for kind, ops in (("row", BULK_ROW_OPS), ("sq", BULK_SQ_OPS)):
        for c in range(N_CHUNKS - 1):
            for off, w in _widths_to_ranges(ops):
                slots.append((kind, c, off, w))
    bulk_len = len(slots)
    c = N_CHUNKS - 1
    for off, w in _widths_to_ranges(TAIL_ROW_OPS):
        slots.append(("row", c, off, w))
    for off, w in _widths_to_ranges(TAIL_SQ_ACT):
        slots.append(("sq", c, off, w))
    return slots, bulk_len


SLOTS, BULK_LEN = _slots()
NSLOT = len(SLOTS)
_SLOT_COL = {(k, c, off): i for i, (k, c, off, _w) in enumerate(SLOTS)}

LAST_RESULTS = None              # BassKernelResults of the most recent run
_NC_CACHE = {}


def _build_bass(bufs=3):
    import concourse.mybir as mybir
    from concourse import bacc
    from concourse.tile import TileContext

    nc = bacc.Bacc(
        "TRN2", target_bir_lowering=False, debug=False, num_devices=N_CORES
    )
    w = nc.declare_dram_parameter("w", [D, COLS], mybir.dt.float32, isOutput=False)
    out = nc.declare_dram_parameter(
        "stats", [P, NSLOT], mybir.dt.float32, isOutput=True
    )

    f32 = mybir.dt.float32
    last_c = N_CHUNKS - 1
    with TileContext(nc) as tc:
        with (
            tc.tile_pool(name="wpool", bufs=bufs) as wpool,
            tc.tile_pool(name="spool", bufs=1) as spool,
            tc.tile_pool(name="scratch", bufs=1) as scpool,
        ):
            stats = spool.tile([P, NSLOT], f32)
            scratch = scpool.tile([P, max(max(BULK_SQ_OPS), max(TAIL_SQ_ACT))], f32)
            for c in range(N_CHUNKS):
                last = c == last_c
                ctile = wpool.tile([P, COLS], f32, tag="wtile")
                for off, f in _widths_to_ranges(TAIL_DMA if last else BULK_DMA):
                    nc.sync.dma_start(
                        out=ctile[:, off:off + f],
                        in_=w[c * P:(c + 1) * P, off:off + f],
                    )
                for off, f in _widths_to_ranges(
                    TAIL_ROW_OPS if last else BULK_ROW_OPS
                ):
                    sr = _SLOT_COL[("row", c, off)]
                    nc.vector.reduce_sum(
                        stats[:, sr:sr + 1],
                        ctile[:, off:off + f],
                        axis=mybir.AxisListType.X,
                    )
                for off, f in _widths_to_ranges(
                    TAIL_SQ_ACT if last else BULK_SQ_OPS
                ):
                    sq = _SLOT_COL[("sq", c, off)]
                    nc.scalar.activation(
                        scratch[:, :f],
                        ctile[:, off:off + f],
                        mybir.ActivationFunctionType.Square,
                        accum_out=stats[:, sq:sq + 1],
                    )
                if c == N_CHUNKS - 2:
                    # bulk stats leave mid-stream on the SP queue
                    nc.sync.dma_start(
                        out=out[:, :BULK_LEN], in_=stats[:, :BULK_LEN]
                    )
            # both engines retire their last op at ~the same time, so one
            # small SP DMA for the final-chunk stats beats per-engine DMAs
            # (each extra DMA pays its own completion latency)
            nc.sync.dma_start(
                out=out[:, BULK_LEN:], in_=stats[:, BULK_LEN:]
            )
    nc.compile()
    return nc


def kernel(softmax_weight, group_ids=None, batch_size=32, **_ignored):
    global LAST_RESULTS
    from concourse.bass_utils import run_bass_kernel_spmd

    W = np.ascontiguousarray(np.asarray(softmax_weight, dtype=np.float32))
    assert W.shape == (D, N_CLASSES), W.shape
    bs = float(np.asarray(batch_size))

    if "nc" not in _NC_CACHE:
        _NC_CACHE["nc"] = _build_bass()
    nc = _NC_CACHE["nc"]

    in_maps = [
        {"w": np.ascontiguousarray(W[:, k * COLS:(k + 1) * COLS])}
        for k in range(N_CORES)
    ]
    LAST_RESULTS = run_bass_kernel_spmd(nc, in_maps, core_ids=list(range(N_CORES)))

    om = 0.0
    t = np.zeros(D, np.float64)
    for r in LAST_RESULTS.results:
        st = r["stats"].astype(np.float64)          # [P, NSLOT]
        for i, (kind, c, _off, _w) in enumerate(SLOTS):
            if kind == "row":
                t[c * P:(c + 1) * P] += st[:, i]
            else:
                om += st[:, i].sum()

    T = (t @ t) / N_CLASSES
    loss = om + 0.5 * (om - T) / bs
    return np.asarray(loss, dtype=np.float32)


# revision 20
# speedup vs baseline: 1.0022x; 1.0022x over previous
"""Trainium2 Bass kernel for nn_ClusterLoss (segment_reduce family).

Reference computes:
    loss = w0*omega_mean + (w1*omega_between + w2*omega_within) / bs
with (w0, w1, w2) = (1.0, 0.5, 0.5).

Algebra: with S_c the per-group column sums, t the total column sum vector,
B = sum_c ||S_c||^2 / max(m_c, 1):
    omega_within  = omega_mean - B
    omega_between = B - ||t||^2 / n
Since w1 == w2, B cancels exactly:
    loss = omega_mean + 0.5*(omega_mean - ||t||^2/n)/bs
so only one streaming pass over W is needed: sum of squares + row sums.
group_ids does not influence the result.

Device plan (per core, column shard [1024, 6250] f32, 8 chunks of 128 rows):
  - chunk-resident SBUF tiles [128 x 6250], filled by 1250-col DMAs
    (descending DMA widths on the last chunk so the stream tail is fine)
  - VectorE (DVE): reduce_sum along free axis -> per-partition row sums
  - ScalarE (ACT): Square activation with accum_out -> per-partition sq sums
  Engine op splits are aligned with DMA arrivals on the last chunk so the
  serial backlog after the final byte is minimal.
  - stats for chunks 0..6 are DMA'd out mid-stream; only the last chunk's
    small stats DMA sits on the critical tail.
Host reduces the tiny [128, NSLOT] per-core stats in float64.
"""

import numpy as np

D = 1024
N_CLASSES = 50000
N_CORES = 8
P = 128
COLS = N_CLASSES // N_CORES      # 6250 columns per core
N_CHUNKS = D // P                # 8 partition chunks

BULK_DMA = (1250, 1250, 1250, 1250, 1250)
TAIL_DMA = (1250, 1250, 1250, 625, 625, 625, 313, 312)
BULK_ROW_OPS = (2500, 2500, 1250)
BULK_SQ_OPS = (2500, 2500, 1250)
TAIL_ROW_OPS = TAIL_DMA                       # DVE rowsums, aligned to DMAs
TAIL_SQ_ACT = (1250, 1250, 1250, 1250, 1250)  # ACT squares: fewer, coarser ops
# (ACT pays ~0.37us fixed cost per accum op, so five 1250-wide ops beat
# DMA-aligned ops in the tail; sq op k is ready once its two 625 DMAs land)
# NOTE: offloading tail squares to DVE via tensor_tensor_reduce looked good in
# the cost model but crashes real TRN2 (NRT_EXEC_UNIT_UNRECOVERABLE) in this
# toolchain even in its canonical broadcast-out form, so all squares stay on ACT.


def _widths_to_ranges(widths, off=0):
    out = []
    for w in widths:
        out.append((off, w))
        off += w
    return out


def _slots():
    """stats-column layout:
      section A (bulk, chunks 0..6): rows then sqs, BULK_OPS splits
      section B (chunk 7): rows (TAIL_ROW_OPS) | sqs (TAIL_SQ_ACT)
    Returns slots: slots[i] = (kind, chunk, off, width)."""
    slots = []
    for kind, ops in (("row", BULK_ROW_OPS), ("sq", BULK_SQ_OPS)):
        for c in range(N_CHUNKS - 1):
            for off, w in _widths_to_ranges(ops):
                slots.append((kind, c, off, w))
    bulk_len = len(slots)
    c = N_CHUNKS - 1
    for off, w in _widths_to_ranges(TAIL_ROW_OPS):
        slots.append(("row", c, off, w))
    for off, w in _widths_to_ranges(TAIL_SQ_ACT):
        slots.append(("sq", c, off, w))
    return slots, bulk_len


SLOTS, BULK_LEN = _slots()
NSLOT = len(SLOTS)
_SLOT_COL = {(k, c, off): i for i, (k, c, off, _w) in enumerate(SLOTS)}

LAST_RESULTS = None              # BassKernelResults of the most recent run
_NC_CACHE = {}


def _build_bass(bufs=3):
    import concourse.mybir as mybir
    from concourse import bacc
    from concourse.tile import TileContext

    nc = bacc.Bacc(
        "TRN2", target_bir_lowering=False, debug=False, num_devices=N_CORES
    )
    w = nc.declare_dram_parameter("w", [D, COLS], mybir.dt.float32, isOutput=False)
    out = nc.declare_dram_parameter(
        "stats", [P, NSLOT], mybir.dt.float32, isOutput=True
    )

    f32 = mybir.dt.float32
    last_c = N_CHUNKS - 1
    with TileContext(nc) as tc:
        with (
            tc.tile_pool(name="wpool", bufs=bufs) as wpool,
            tc.tile_pool(name="spool", bufs=1) as spool,
            tc.tile_pool(name="scratch", bufs=1) as scpool,
        ):
            stats = spool.tile([P, NSLOT], f32)
            scratch = scpool.tile([P, max(max(BULK_SQ_OPS), max(TAIL_SQ_ACT))], f32)
            for c in range(N_CHUNKS):
                last = c == last_c
                ctile = wpool.tile([P, COLS], f32, tag="wtile")
                for off, f in _widths_to_ranges(TAIL_DMA if last else BULK_DMA):
                    nc.sync.dma_start(
                        out=ctile[:, off:off + f],
                        in_=w[c * P:(c + 1) * P, off:off + f],
                    )
                if last:
                    # bulk stats drain in the free DMA slot behind the last
                    # w-DMAs (deps: chunks 0..6 compute, long since done).
                    # Emitting this any earlier displaces the w-stream on the
                    # exclusive DMA resource by its transfer time.
                    nc.sync.dma_start(
                        out=out[:, :BULK_LEN], in_=stats[:, :BULK_LEN]
                    )
                for off, f in _widths_to_ranges(
                    TAIL_ROW_OPS if last else BULK_ROW_OPS
                ):
                    sr = _SLOT_COL[("row", c, off)]
                    nc.vector.reduce_sum(
                        stats[:, sr:sr + 1],
                        ctile[:, off:off + f],
                        axis=mybir.AxisListType.X,
                    )
                for off, f in _widths_to_ranges(
                    TAIL_SQ_ACT if last else BULK_SQ_OPS
                ):
                    sq = _SLOT_COL[("sq", c, off)]
                    nc.scalar.activation(
                        scratch[:, :f],
                        ctile[:, off:off + f],
                        mybir.ActivationFunctionType.Square,
                        accum_out=stats[:, sq:sq + 1],
                    )
            # both engines retire their last op at ~the same time, so one
            # small SP DMA for the final-chunk stats beats per-engine DMAs
            # (each extra DMA pays its own completion latency)
            nc.sync.dma_start(
                out=out[:, BULK_LEN:], in_=stats[:, BULK_LEN:]
            )
    nc.compile()
    return nc


def kernel(softmax_weight, group_ids=None, batch_size=32, **_ignored):
    global LAST_RESULTS
    from concourse.bass_utils import run_bass_kernel_spmd

    W = np.ascontiguousarray(np.asarray(softmax_weight, dtype=np.float32))
    assert W.shape == (D, N_CLASSES), W.shape
    bs = float(np.asarray(batch_size))

    if "nc" not in _NC_CACHE:
        _NC_CACHE["nc"] = _build_bass()
    nc = _NC_CACHE["nc"]

    in_maps = [
        {"w": np.ascontiguousarray(W[:, k * COLS:(k + 1) * COLS])}
        for k in range(N_CORES)
    ]
    LAST_RESULTS = run_bass_kernel_spmd(nc, in_maps, core_ids=list(range(N_CORES)))

    om = 0.0
    t = np.zeros(D, np.float64)
    for r in LAST_RESULTS.results:
        st = r["stats"].astype(np.float64)          # [P, NSLOT]
        for i, (kind, c, _off, _w) in enumerate(SLOTS):
            if kind == "row":
                t[c * P:(c + 1) * P] += st[:, i]
            else:
                om += st[:, i].sum()

    T = (t @ t) / N_CLASSES
    loss = om + 0.5 * (om - T) / bs
    return np.asarray(loss, dtype=np.float32)
